# revision 2
# baseline (speedup 1.0000x reference)
"""Multi-head self-attention (no causal mask) on 8 Trainium2 NeuronCores, v2.

Problem: B=2, S=2048, D=768, H=12 heads (head_dim 64), fp32.
Sharding: batch x head-group. Core c handles batch c//4 and heads
3*(c%4) .. 3*(c%4)+2 (Megatron column-parallel QKV, row-parallel Wo).
Host sums the 4 partials per batch and adds bo.

v2 design (vs v1): head-sequential attention with a transposed attn@V
("orientation b") so the softmax probabilities p^T feed matmuls as lhsT
slices and accumulate [q,d]-natural outputs directly in PSUM across all
16 k-tiles; denominators ride along as a ones-column of V (N=65). The
p/V/attn-out/Wo path runs in bf16 (halves PE rows + output in natural
layout); Q/K/scores stay f32r. Scores self-pair each head across
consecutive k-tiles on PE row groups 0/64 (Q^T/K^T duplicated at both
partition bases). Exp on ScalarE is the projected bottleneck (~96
instructions of [128,1024]); emission order keeps it fed from ~9us in.
"""

import numpy as np

_CACHE = {}

S = 2048
D = 768
HLOC = 3          # heads per core
NKT = 6           # 768 / 128 d-tiles
NST = 16          # 2048 / 128 s-tiles


def _emit(nc, tc, ctx, dram, loop_n=None):
    import concourse.bass as bass
    import concourse.mybir as mybir
    from concourse.masks import make_identity

    f32 = mybir.dt.float32
    f32r = mybir.dt.float32r
    bf16 = mybir.dt.bfloat16
    add = mybir.AluOpType.add
    mult = mybir.AluOpType.mult
    Exp = mybir.ActivationFunctionType.Exp

    xb, wq, wk, wv, wo, bq, bk, bv, out_d = (
        dram["xb"], dram["wq"], dram["wk"], dram["wv"], dram["wo"],
        dram["bq"], dram["bk"], dram["bv"], dram["out"],
    )

    consts = ctx.enter_context(tc.tile_pool(name="consts", bufs=1))
    xpool = ctx.enter_context(tc.tile_pool(name="xpool", bufs=6))
    ppool = ctx.enter_context(tc.tile_pool(name="ppool", bufs=22))
    rpool = ctx.enter_context(tc.tile_pool(name="rpool", bufs=2))
    opool = ctx.enter_context(tc.tile_pool(name="opool", bufs=3))
    pab = ctx.enter_context(tc.tile_pool(name="pab", bufs=2, space="PSUM"))
    pss = ctx.enter_context(tc.tile_pool(name="pss", bufs=2, space="PSUM"))
    pacc = ctx.enter_context(tc.tile_pool(name="pacc", bufs=2, space="PSUM"))

    # ---- constants / persistent tensors ----
    ident = consts.tile([128, 128], f32)
    make_identity(nc, ident)
    ident_r = consts.tile([128, 128], f32r)
    nc.vector.tensor_copy(out=ident_r, in_=ident)
    ident_bf = consts.tile([128, 128], bf16)
    nc.vector.tensor_copy(out=ident_bf, in_=ident)

    xt = consts.tile([128, NKT, S], bf16)            # x^T
    # Q^T/K^T per head at BOTH partition bases for score self-pairing:
    # slot0 = [A;B], slot1 = [B;A], slot2 = [C;C]
    qt = consts.tile([128, 3, S], bf16)
    kt_ = consts.tile([128, 3, S], bf16)
    v_sb = consts.tile([128, NST, HLOC, 66], bf16)   # V natural + ones col @64
    attn_nat = consts.tile([128, NST, 192], bf16)    # normalized attn out [q,d]
    attnT1 = consts.tile([128, NST, 128], bf16)      # attn_out^T rows 0:128
    attnT2 = consts.tile([128, NST // 2, 128], bf16)  # rows 128:192, alt bases

    w_qsb = consts.tile([128, NKT, 128], bf16)
    w_ksb = consts.tile([128, NKT, 128], bf16)
    w_qkc = consts.tile([128, NKT, 128], bf16)  # [Wq_C | Wk_C]
    w_vsb = consts.tile([128, NKT, 192], bf16)
    wo_sb = consts.tile([128, 2, D], bf16)
    bq1 = consts.tile([128, 1], f32)
    bq2 = consts.tile([64, 1], f32)
    bk1 = consts.tile([128, 1], f32)
    bkC = consts.tile([128, 1], f32)
    bv_bc = consts.tile([128, 192], f32)

    # ---- weight / bias loads (outside any timing loop) ----
    wq_r = wq.rearrange("(t p) c -> p t c", p=128)
    wk_r = wk.rearrange("(t p) c -> p t c", p=128)
    wv_r = wv.rearrange("(t p) c -> p t c", p=128)
    # stage f32 weights through xpool tiles, cast to bf16 working copies
    for dst, src in ((w_qsb, wq_r[:, :, 0:128]), (w_ksb, wk_r[:, :, 0:128])):
        st_t = xpool.tile([128, NKT, 128], f32, name="wst", tag="x")
        nc.sync.dma_start(out=st_t, in_=src)
        nc.vector.tensor_copy(out=dst, in_=st_t)
    st_t = xpool.tile([128, NKT, 128], f32, name="wst_c", tag="x")
    nc.sync.dma_start(out=st_t[:, :, 0:64], in_=wq_r[:, :, 128:192])
    nc.sync.dma_start(out=st_t[:, :, 64:128], in_=wk_r[:, :, 128:192])
    nc.vector.tensor_copy(out=w_qkc, in_=st_t)
    for hf in range(2):
        st_t = xpool.tile([128, NKT, 96], f32, name=f"wst_v{hf}", tag="x")
        nc.sync.dma_start(out=st_t, in_=wv_r[:, :, hf * 96:(hf + 1) * 96])
        nc.vector.tensor_copy(out=w_vsb[:, :, hf * 96:(hf + 1) * 96], in_=st_t)
    for sl, rows in ((0, wo[0:128, :]), (1, None)):
        st_t = xpool.tile([128, D], f32, name=f"wst_o{sl}", tag="x")
        if sl == 0:
            nc.sync.dma_start(out=st_t, in_=rows)
        else:
            nc.sync.dma_start(out=st_t[0:64, :], in_=wo[128:192, :])
            nc.sync.dma_start(out=st_t[64:128, :], in_=wo[128:192, :])
        nc.vector.tensor_copy(out=wo_sb[:, sl, :], in_=st_t)
    nc.sync.dma_start(out=bq1, in_=bq[0:128].rearrange("(p o) -> p o", o=1))
    nc.sync.dma_start(out=bq2, in_=bq[128:192].rearrange("(p o) -> p o", o=1))
    nc.sync.dma_start(out=bk1, in_=bk[0:128].rearrange("(p o) -> p o", o=1))
    nc.sync.dma_start(out=bkC[64:128, :],
                      in_=bk[128:192].rearrange("(p o) -> p o", o=1))
    bv_b = bass.AP(tensor=bv.tensor, offset=bv.offset, ap=[[0, 128]] + list(bv.ap))
    nc.sync.dma_start(out=bv_bc, in_=bv_b)
    ones_bf = consts.tile([128, NST * HLOC], bf16)
    nc.vector.memset(ones_bf, 1.0)
    nc.vector.tensor_copy(
        out=v_sb[:, :, :, 64:65],
        in_=ones_bf.rearrange("p (a b c) -> p a b c", b=HLOC, c=1))

    # head -> (slot at base 0, slot at base 64)
    HSLOT = [(0, 1), (1, 0), (2, 2)]
    p_tiles = {}          # (h, half, kt) -> exp'd scores tile [128,1024] bf16
    acc_tiles = {}        # (h, half, quarter) -> PSUM [128, 4, 128] f32

    def qsl(g):
        return slice(g * 512, (g + 1) * 512)

    def emit_x(g):
        xg = []
        for j in range(4):
            st = 4 * g + j
            x_t = xpool.tile([128, D], f32r, name=f"x_{st}", tag="x")
            nc.sync.dma_start(out=x_t, in_=xb[st * 128:(st + 1) * 128, :])
            xg.append(x_t)
        for dt in range(NKT):
            ptr = pab.tile([128, 512], f32r, tag="mm", name=f"pt_{g}_{dt}")
            for j in range(4):
                nc.tensor.transpose(ptr[:, j * 128:(j + 1) * 128],
                                    xg[j][:, dt * 128:(dt + 1) * 128], ident_r)
            nc.vector.tensor_copy(out=xt[:, dt, qsl(g)], in_=ptr)

    def emit_qk(g):
        # Q/K projections for q-chunk g: heads A,B packed (M=128), then
        # swapped-base duplicates for score self-pairing.
        for dst, wsb, b1, sc in ((qt, w_qsb, bq1, 0.125),
                                 (kt_, w_ksb, bk1, None)):
            pp = pab.tile([128, 512], f32, tag="mm", name=f"pp_{g}")
            for kti in range(NKT):
                nc.tensor.matmul(pp, lhsT=wsb[:, kti, :],
                                 rhs=xt[:, kti, qsl(g)],
                                 start=(kti == 0), stop=(kti == NKT - 1))
            if sc is None:
                nc.vector.tensor_scalar_add(dst[:, 0, qsl(g)], pp, b1)
            else:
                nc.vector.tensor_scalar(dst[:, 0, qsl(g)], pp, b1, sc,
                                        add, mult)
            nc.vector.tensor_copy(out=dst[64:128, 1, qsl(g)],
                                  in_=dst[0:64, 0, qsl(g)])
            nc.vector.tensor_copy(out=dst[0:64, 1, qsl(g)],
                                  in_=dst[64:128, 0, qsl(g)])

    def emit_c(g):
        # head C: merged [Q_C | K_C] projection, duplicated to both bases.
        pp2 = pab.tile([128, 512], f32, tag="mm", name=f"pp2_{g}")
        for kti in range(NKT):
            nc.tensor.matmul(pp2, lhsT=w_qkc[:, kti, :],
                             rhs=xt[:, kti, qsl(g)],
                             start=(kti == 0), stop=(kti == NKT - 1))
        nc.vector.tensor_scalar(qt[0:64, 2, qsl(g)], pp2[0:64, :],
                                bq2, 0.125, add, mult)
        nc.vector.tensor_scalar_add(kt_[64:128, 2, qsl(g)],
                                    pp2[64:128, :], bkC[64:128, :])
        nc.vector.tensor_copy(out=qt[64:128, 2, qsl(g)],
                              in_=qt[0:64, 2, qsl(g)])
        nc.vector.tensor_copy(out=kt_[0:64, 2, qsl(g)],
                              in_=kt_[64:128, 2, qsl(g)])

    def emit_v(g):
        for j in range(4):
            st = 4 * g + j
            pv = pab.tile([128, 192], f32, tag="mm", name=f"pv_{st}")
            for kti in range(NKT):
                nc.tensor.matmul(pv,
                                 lhsT=xt[:, kti, st * 128:(st + 1) * 128],
                                 rhs=w_vsb[:, kti, :],
                                 start=(kti == 0), stop=(kti == NKT - 1))
            nc.vector.tensor_tensor(
                out=v_sb[:, st, :, 0:64],
                in0=pv.rearrange("p (h d) -> p h d", h=3),
                in1=bv_bc.rearrange("p (h d) -> p h d", h=3),
                op=add)

    def emit_scores(h, half, kt_pair):
        # one self-paired k-tile pair: kt even at rows 0-63, kt+1 at 64-127
        s0, s1 = HSLOT[h]
        ktE, ktO = kt_pair, kt_pair + 1
        psE = pss.tile([128, 1024], f32, tag="ps", name=f"ps_{h}_{half}_{ktE}")
        psO = pss.tile([128, 1024], f32, tag="ps", name=f"ps_{h}_{half}_{ktO}")
        # c-chunks back-to-back per k-tile so the weight load is reused
        # (dedupe); E/O still co-execute on PE row groups 0/64.
        for base, slot, kti, ps in ((0, s0, ktE, psE), (64, s1, ktO, psO)):
            for c in range(2):
                qs = slice(half * 1024 + c * 512, half * 1024 + (c + 1) * 512)
                cs = slice(c * 512, (c + 1) * 512)
                nc.tensor.matmul(
                    ps[:, cs],
                    lhsT=kt_[base:base + 64, slot, kti * 128:(kti + 1) * 128],
                    rhs=qt[base:base + 64, slot, qs], start=True, stop=True)
        for kti, ps in ((ktE, psE), (ktO, psO)):
            p_t = ppool.tile([128, 1024], bf16, tag="p",
                             name=f"p_{h}_{half}_{kti}")
            nc.scalar.activation(out=p_t, in_=ps, func=Exp)
            p_tiles[(h, half, kti)] = p_t

    def emit_attnv(h, half, kts):
        for q in range(2):
            if (h, half, q) not in acc_tiles:
                acc_tiles[(h, half, q)] = pacc.tile(
                    [128, 4, 128], f32, tag="acc", name=f"acc_{h}_{half}_{q}")
        for kti in kts:
            p_t = p_tiles.pop((h, half, kti))
            for ql in range(8):
                acc = acc_tiles[(h, half, ql // 4)]
                # start=True zeroes the WHOLE psum bank, so open each
                # bank-quartet exactly once (first slot of first k-tile).
                nc.tensor.matmul(
                    acc[:, ql % 4, 0:65],
                    lhsT=p_t[:, ql * 128:(ql + 1) * 128],
                    rhs=v_sb[:, kti, h, 0:65],
                    start=(kti == 0 and ql % 4 == 0), stop=(kti == 15),
                    skip_group_check=True)

    def emit_norm(h, half):
        for ql in range(8):
            qt_i = half * 8 + ql
            acc = acc_tiles[(h, half, ql // 4)][:, ql % 4, :]
            r_t = rpool.tile([128, 1], f32, name=f"r_{h}_{qt_i}", tag="r")
            nc.vector.reciprocal(out=r_t, in_=acc[:, 64:65])
            nc.vector.tensor_scalar_mul(
                attn_nat[:, qt_i, h * 64:(h + 1) * 64], acc[:, 0:64], r_t)
        acc_tiles.pop((h, half, 0))
        acc_tiles.pop((h, half, 1))

    def emit_out(j):
        # output step for q-tile pair (2j, 2j+1): transposes + Wo + DMA
        qts = (2 * j, 2 * j + 1)
        for qt_i in qts:
            tr1 = pab.tile([128, 128], bf16, tag="mm", name=f"tr1_{qt_i}")
            nc.tensor.transpose(tr1, attn_nat[:, qt_i, 0:128], ident_bf)
            nc.vector.tensor_copy(out=attnT1[:, qt_i, :], in_=tr1)
            tr2 = pab.tile([64, 128], bf16, tag="mm", name=f"tr2_{qt_i}")
            nc.tensor.transpose(tr2, attn_nat[:, qt_i, 128:192], ident_bf)
            b = 64 * (qt_i % 2)
            nc.vector.tensor_copy(out=attnT2[b:b + 64, j, :], in_=tr2)
        o_ts = {}
        for qt_i in qts:
            o_ts[qt_i] = opool.tile([128, D], f32, name=f"o_{qt_i}", tag="o")
        for e in range(2):
            esl = slice(e * 384, (e + 1) * 384)
            pws = {}
            for qt_i in qts:
                pw = pab.tile([128, 512], f32, tag="mm", name=f"pw_{qt_i}_{e}")
                nc.tensor.matmul(pw[:, 0:384], lhsT=attnT1[:, qt_i, :],
                                 rhs=wo_sb[:, 0, esl], start=True, stop=False)
                pws[qt_i] = pw
            for qt_i in qts:
                b = 64 * (qt_i % 2)
                nc.tensor.matmul(pws[qt_i][:, 0:384],
                                 lhsT=attnT2[b:b + 64, j, :],
                                 rhs=wo_sb[b:b + 64, 1, esl],
                                 start=False, stop=True)
            for qt_i in qts:
                nc.vector.tensor_copy(out=o_ts[qt_i][:, esl],
                                      in_=pws[qt_i][:, 0:384])
        for qt_i in qts:
            nc.sync.dma_start(out=out_d[qt_i * 128:(qt_i + 1) * 128, :],
                              in_=o_ts[qt_i])

    def body():
        # prologue: x^T + Q/K for all 4 groups (C projections deferred)
        for g in range(4):
            emit_x(g)
            emit_qk(g)
        # ---- half 0 ----
        # A: scores only (attnV deferred so exp stream starts ASAP)
        for kp in range(0, 16, 2):
            emit_scores(0, 0, kp)
        # B: scores + V projections + A attnV
        for i, kp in enumerate(range(0, 16, 2)):
            emit_scores(1, 0, kp)
            if i < 4:
                emit_v(i)
            emit_attnv(0, 0, (kp, kp + 1))
        # C: C projections + scores + B attnV; A normalize first.
        # C scores need q-chunks g0,g1 (half 0) up front; key chunks g2,g3
        # are only touched from kp=8 on.
        emit_norm(0, 0)
        emit_c(0)
        emit_c(1)
        for i, kp in enumerate(range(0, 16, 2)):
            if i < 2:
                emit_c(2 + i)
            emit_scores(2, 0, kp)
            emit_attnv(1, 0, (kp, kp + 1))
        emit_norm(1, 0)
        # ---- half 1 ----
        for i, kp in enumerate(range(0, 16, 2)):
            emit_scores(0, 1, kp)
            emit_attnv(2, 0, (kp, kp + 1))
        emit_norm(2, 0)
        for i, kp in enumerate(range(0, 16, 2)):
            emit_scores(1, 1, kp)
            emit_attnv(0, 1, (kp, kp + 1))
            if i % 2 == 0:
                emit_out(i // 2)          # half-0 output pairs j=0..3
        emit_norm(0, 1)
        for i, kp in enumerate(range(0, 16, 2)):
            emit_scores(2, 1, kp)
            emit_attnv(1, 1, (kp, kp + 1))
        emit_norm(1, 1)
        for kp in range(0, 16, 2):
            emit_attnv(2, 1, (kp, kp + 1))
        emit_norm(2, 1)
        for j in range(4, 8):
            emit_out(j)

    if loop_n is None:
        body()
    else:
        with tc.For_i(0, loop_n, 1):
            body()


def _build(loop_n=None):
    from contextlib import ExitStack

    import concourse.bacc as bacc
    import concourse.mybir as mybir
    import concourse.tile as tile

    f32 = mybir.dt.float32
    f32r = mybir.dt.float32r
    nc = bacc.Bacc("TRN2", target_bir_lowering=False, debug=False, num_devices=8)
    dram = {
        "xb": nc.dram_tensor("xb", [S, D], f32r, kind="ExternalInput").ap(),
        "wq": nc.dram_tensor("wq", [D, 192], f32, kind="ExternalInput").ap(),
        "wk": nc.dram_tensor("wk", [D, 192], f32, kind="ExternalInput").ap(),
        "wv": nc.dram_tensor("wv", [D, 192], f32, kind="ExternalInput").ap(),
        "wo": nc.dram_tensor("wo", [192, D], f32, kind="ExternalInput").ap(),
        "bq": nc.dram_tensor("bq", [192], f32, kind="ExternalInput").ap(),
        "bk": nc.dram_tensor("bk", [192], f32, kind="ExternalInput").ap(),
        "bv": nc.dram_tensor("bv", [192], f32, kind="ExternalInput").ap(),
        "out": nc.dram_tensor("out", [S, D], f32, kind="ExternalOutput").ap(),
    }
    with tile.TileContext(nc) as tc:
        with ExitStack() as ctx:
            _emit(nc, tc, ctx, dram, loop_n=loop_n)
    nc.compile()
    return nc


def _get_nc():
    if "nc" not in _CACHE:
        _CACHE["nc"] = _build()
    return _CACHE["nc"]


def _shard(inputs):
    x = np.asarray(inputs["x"], np.float32)
    Wq = np.asarray(inputs["Wq"], np.float32)
    Wk = np.asarray(inputs["Wk"], np.float32)
    Wv = np.asarray(inputs["Wv"], np.float32)
    Wo = np.asarray(inputs["Wo"], np.float32)
    bq = np.asarray(inputs["bq"], np.float32)
    bk = np.asarray(inputs["bk"], np.float32)
    bv = np.asarray(inputs["bv"], np.float32)
    in_maps = []
    for c in range(8):
        b, g = divmod(c, 4)
        o = 192 * g
        in_maps.append({
            "xb": np.ascontiguousarray(x[b]),
            "wq": np.ascontiguousarray(Wq[:, o:o + 192]),
            "wk": np.ascontiguousarray(Wk[:, o:o + 192]),
            "wv": np.ascontiguousarray(Wv[:, o:o + 192]),
            "wo": np.ascontiguousarray(Wo[o:o + 192, :]),
            "bq": np.ascontiguousarray(bq[o:o + 192]),
            "bk": np.ascontiguousarray(bk[o:o + 192]),
            "bv": np.ascontiguousarray(bv[o:o + 192]),
        })
    return in_maps


def kernel(x, Wq, bq, Wk, bk, Wv, bv, Wo, bo):
    from concourse.bass_utils import run_bass_kernel_spmd

    nc = _get_nc()
    in_maps = _shard(dict(x=x, Wq=Wq, Wk=Wk, Wv=Wv, Wo=Wo,
                          bq=bq, bk=bk, bv=bv))
    res = run_bass_kernel_spmd(nc, in_maps, core_ids=list(range(8)))
    out = np.zeros((2, S, D), np.float32)
    for c in range(8):
        out[c // 4] += res.results[c]["out"]
    out += np.asarray(bo, np.float32)
    return out
